# revision 1
# baseline (speedup 1.0000x reference)
"""GAT + global-max-pool + LSTM + Linear kernel for Trainium2 (8 NeuronCores).

Sharding: data-parallel over the batch axis B=8 -> one sequence b per core.
Each core computes the GAT over its 16 graphs (t=0..15), global-max-pools,
runs the LSTM over its sequence locally, and emits one [8] output row.

GAT aggregation strategy (per core, per graph g, head h):
  z[m, n]   = a_s[m] + a_d[n]                 (dense outer sum via PE broadcast)
  ex[m, n]  = exp(leaky_relu(z)) = max(exp(z), exp(0.2 z))   (two ACT Exp passes)
  A[m, n]   = ex * cnt[m, n]                  (cnt = host-built edge-count mask)
  out[n, f] = sum_m xp[m, f] A[m, n] ;  denom[n] = sum_m A[m, n]
  (single matmul per 128-node src block: lhsT = [xp_h | ones] -> 33 out rows)
  gat[n, f] = relu(out[n, f] / denom[n] + b_gat); pooled = max over n.

softmax max-subtraction is dropped: alpha = ex/sum(ex) is invariant to the
per-dst shift and fp32/bf16 exp() of |z| <~ 10 cannot overflow.
"""

import numpy as np

import concourse.bacc as bacc
import concourse.bass as bass
import concourse.mybir as mybir
import concourse.tile as tile
from concourse.bass_utils import run_bass_kernel_spmd

B, T, N, F_IN = 8, 16, 1000, 16
H, D = 4, 32
HD = H * D          # 128
HL = 64
OUT = 8
NEG = 0.2
NPAD = 1024         # padded node count
NBLK = 8            # src blocks of 128
G = T               # graphs per core

FP = mybir.dt.float32
BF = mybir.dt.bfloat16
AX = mybir.AxisListType
AF = mybir.ActivationFunctionType
OPS = mybir.AluOpType

_CACHE = {}


def _build_nc():
    nc = bacc.Bacc("TRN2", target_bir_lowering=False, debug=False)

    # ---- DRAM I/O ----
    d_x = nc.dram_tensor("x_in", [G * NPAD, F_IN], FP, kind="ExternalInput").ap()
    d_wgat = nc.dram_tensor("w_gat", [F_IN, HD], FP, kind="ExternalInput").ap()
    d_wa8 = nc.dram_tensor("w_a8", [F_IN, 128], FP, kind="ExternalInput").ap()
    d_cnt = nc.dram_tensor("cntmask", [128, NBLK * NPAD], BF, kind="ExternalInput").ap()
    d_ones = nc.dram_tensor("ones_row", [1, 128], BF, kind="ExternalInput").ap()
    d_ones32 = nc.dram_tensor("ones_c32", [33, 32], FP, kind="ExternalInput").ap()
    d_ident = nc.dram_tensor("ident", [128, 128], FP, kind="ExternalInput").ap()
    d_bgat = nc.dram_tensor("b_gat", [32, H], FP, kind="ExternalInput").ap()
    d_wih = nc.dram_tensor("wih_t", [HD, 4 * HL], FP, kind="ExternalInput").ap()
    d_whh = nc.dram_tensor("whh_t", [HL, 4 * HL], FP, kind="ExternalInput").ap()
    d_bls = nc.dram_tensor("b_lstm", [HL, 4], FP, kind="ExternalInput").ap()
    d_wclf = nc.dram_tensor("wclf_t", [HL, OUT], FP, kind="ExternalInput").ap()
    d_bclf = nc.dram_tensor("b_clf", [OUT, 1], FP, kind="ExternalInput").ap()
    d_y = nc.dram_tensor("y", [OUT, 1], FP, kind="ExternalOutput").ap()

    with tile.TileContext(nc) as tc:
        with (
            tc.tile_pool(name="const", bufs=1) as cpool,
            tc.tile_pool(name="stage", bufs=2) as spool,
            tc.tile_pool(name="edense", bufs=3) as epool,
            tc.tile_pool(name="small", bufs=2) as mpool,
            tc.tile_pool(name="lstm", bufs=2) as lpool,
            tc.tile_pool(name="ps_misc", bufs=2, space="PSUM") as ps_misc,
            tc.tile_pool(name="ps_out", bufs=1, space="PSUM") as ps_out,
            tc.tile_pool(name="ps_vb", bufs=1, space="PSUM") as ps_vb,
        ):
            # ---- load constants ----
            c_x = cpool.tile([128, G * NPAD * F_IN // 128], FP, tag="xall")  # [128, 2048]
            # x rows (t*1024+m); partition p holds rows {128*i+p}: col block i
            nc.sync.dma_start(
                c_x[:].rearrange("p (i f) -> p i f", f=F_IN),
                d_x.rearrange("(i p) f -> p i f", p=128),
            )
            c_wgat = cpool.tile([F_IN, HD], FP, tag="wgat")
            nc.sync.dma_start(c_wgat[:], d_wgat)
            # col 32h = W_ad[:,h]; col 32h+1 = W_as[:,h]; rest zero
            c_wa8 = cpool.tile([F_IN, 128], FP, tag="wa8")
            nc.sync.dma_start(c_wa8[:], d_wa8)
            c_cnt = cpool.tile([128, NBLK * NPAD], BF, tag="cnt")
            nc.sync.dma_start(c_cnt[:], d_cnt)
            c_id = cpool.tile([128, 128], FP, tag="ident")
            nc.sync.dma_start(c_id[:], d_ident)
            c_onesB = cpool.tile([65, 128], BF, tag="onesB")
            nc.sync.dma_start(c_onesB[0:1, :], d_ones)
            nc.sync.dma_start(c_onesB[32:33, :], d_ones)
            nc.sync.dma_start(c_onesB[64:65, :], d_ones)
            c_ones32 = cpool.tile([33, 32], FP, tag="ones32")
            nc.sync.dma_start(c_ones32[:], d_ones32)
            c_bgat = cpool.tile([32, H], FP, tag="bgat")
            nc.sync.dma_start(c_bgat[:], d_bgat)
            c_wih = cpool.tile([HD, 4 * HL], FP, tag="wih")
            nc.sync.dma_start(c_wih[:], d_wih)
            c_whh = cpool.tile([HL, 4 * HL], FP, tag="whh")
            nc.sync.dma_start(c_whh[:], d_whh)
            c_bls = cpool.tile([HL, 4], FP, tag="bls")
            nc.sync.dma_start(c_bls[:], d_bls)
            c_wclf = cpool.tile([HL, OUT], FP, tag="wclf")
            nc.sync.dma_start(c_wclf[:], d_wclf)
            c_bclf = cpool.tile([OUT, 1], FP, tag="bclf")
            nc.sync.dma_start(c_bclf[:], d_bclf)

            # persistent per-core accumulators
            # u = exp(a_s), u5 = exp(0.2 a_s) column tiles (scale operands);
            # col index = ((g*NBLK + J)*H + h); partition = m within block
            c_uT = cpool.tile([128, G * NBLK * H], FP, tag="uT")
            c_u5T = cpool.tile([128, G * NBLK * H], FP, tag="u5T")
            c_pool = cpool.tile([HD, G], FP, tag="pooled")
            c_ph = []
            for h in range(H):
                ph_tile = cpool.tile([32, G], FP, tag=f"pool{h}")
                c_ph.append(ph_tile)

            for g in range(G):
                # ---- stage A: transpose x_g -> xT [16, 1024] ----
                xT = spool.tile([F_IN, NPAD], FP, tag="xT")
                for j in range(NPAD // 128):
                    i = g * (NPAD // 128) + j   # global 128-row tile index
                    pT = ps_misc.tile([F_IN, 128], FP, tag="pm")
                    nc.tensor.transpose(
                        pT[:], c_x[:, i * F_IN:(i + 1) * F_IN], c_id[:]
                    )
                    nc.vector.tensor_copy(xT[:, j * 128:(j + 1) * 128], pT[:])

                # ---- stage B: xp^T = W_gat^T @ xT ; a8 = W_a8^T @ xT ----
                xpT = spool.tile([HD, NPAD], FP, tag="xpT")
                for half in range(2):
                    pm = ps_misc.tile([HD, 512], FP, tag="pm")
                    nc.tensor.matmul(
                        pm[:], c_wgat[:], xT[:, half * 512:(half + 1) * 512],
                        start=True, stop=True,
                    )
                    nc.vector.tensor_copy(xpT[:, half * 512:(half + 1) * 512], pm[:])
                # v = exp(a_d), v5 = exp(0.2 a_d); a_d for head h on partition 32h
                s8v = spool.tile([128, NPAD], BF, tag="s8v")
                s8v5 = spool.tile([128, NPAD], BF, tag="s8v5")
                for half in range(2):
                    pa = ps_misc.tile([128, 512], FP, tag="pm")
                    nc.tensor.matmul(
                        pa[:], c_wa8[:], xT[:, half * 512:(half + 1) * 512],
                        start=True, stop=True,
                    )
                    nc.scalar.activation(
                        s8v[:, half * 512:(half + 1) * 512], pa[:], AF.Exp,
                        scale=1.0,
                    )
                    nc.scalar.activation(
                        s8v5[:, half * 512:(half + 1) * 512], pa[:], AF.Exp,
                        scale=NEG,
                    )
                # head 3 lives on partition 96 (invalid matmul base): relocate
                s8x3 = spool.tile([1, NPAD], BF, tag="s8x3")
                nc.sync.dma_start(s8x3[:], s8v[96:97, :])
                s8x53 = spool.tile([1, NPAD], BF, tag="s8x53")
                nc.sync.dma_start(s8x53[:], s8v5[96:97, :])

                # xp33: per src block J: [xp_h | 1] column groups, bf16
                # layout [128, NBLK * (H*33)]; col = J*132 + h*33 + d (d<32), ones at h*33+32
                xp33 = spool.tile([128, NBLK * (H * 33 + 4)], BF, tag="xp33")
                for J in range(NBLK):
                    pX = ps_misc.tile([128, 128], FP, tag="pm")
                    nc.tensor.transpose(
                        pX[:], xpT[:, J * 128:(J + 1) * 128], c_id[:]
                    )
                    base = J * (H * 33 + 4)
                    # copy [128, 32] per head into strided slots
                    nc.vector.tensor_copy(
                        xp33[:, base:base + H * 33].rearrange(
                            "p (h q) -> p h q", q=33
                        )[:, :, 0:32],
                        pX[:].rearrange("p (h q) -> p h q", q=32),
                    )
                    nc.vector.memset(
                        xp33[:, base:base + H * 33].rearrange(
                            "p (h q) -> p h q", q=33
                        )[:, :, 32:33],
                        1.0,
                    )
                    # a_sT columns for this (g, J): [128 m, 128] = xT_blk^T @ W_a8
                    pS = ps_misc.tile([128, 128], FP, tag="pm")
                    nc.tensor.matmul(
                        pS[:], xT[:, J * 128:(J + 1) * 128], c_wa8[:],
                        start=True, stop=True,
                    )
                    col = (g * NBLK + J) * H
                    aps = pS[:].rearrange("p (h q) -> p h q", q=32)[:, :, 1:2]
                    nc.scalar.activation(
                        c_uT[:, col:col + H],
                        aps, AF.Exp, scale=1.0,
                    )
                    nc.scalar.activation(
                        c_u5T[:, col:col + H],
                        aps, AF.Exp, scale=NEG,
                    )

                # ---- stage C/D: dense attention + aggregation per head ----
                for h in range(H):
                    # broadcast v rows to all 128 partitions via PE ones-matmul
                    if h < 3:
                        r = 32 * h
                        vrow, v5row = s8v[r:r + 1, :], s8v5[r:r + 1, :]
                    else:
                        r = 0
                        vrow, v5row = s8x3[:], s8x53[:]
                    vB = ps_vb.tile([128, NPAD], FP, tag="vB")
                    v5B = ps_vb.tile([128, NPAD], FP, tag="v5B")
                    for half in range(2):
                        sl = slice(half * 512, (half + 1) * 512)
                        nc.tensor.matmul(
                            vB[:, sl], c_onesB[r:r + 1, :], vrow[:, sl],
                            start=True, stop=True,
                        )
                        nc.tensor.matmul(
                            v5B[:, sl], c_onesB[r:r + 1, :], v5row[:, sl],
                            start=True, stop=True,
                        )
                    oph = ps_out.tile([33, NPAD], FP, tag="oph")
                    for J in range(NBLK):
                        col = (g * NBLK + J) * H + h
                        # E = u[m] * v[n] = exp(a_s[m] + a_d[n])
                        tE = epool.tile([128, NPAD], BF, tag="tE")
                        nc.scalar.activation(
                            tE[:], vB[:], AF.Copy,
                            bias=0.0, scale=c_uT[:, col:col + 1],
                        )
                        # M = max(E, u5[m] * v5[n]) = exp(lrelu(z))
                        tM = epool.tile([128, NPAD], BF, tag="tM")
                        nc.vector.scalar_tensor_tensor(
                            tM[:], v5B[:], c_u5T[:, col:col + 1], tE[:],
                            OPS.mult, OPS.max,
                        )
                        tA = epool.tile([128, NPAD], BF, tag="tA")
                        eng = nc.vector if J < 5 else nc.gpsimd
                        eng.tensor_tensor(
                            tA[:], tM[:], c_cnt[:, J * NPAD:(J + 1) * NPAD], OPS.mult
                        )
                        base = J * (H * 33 + 4) + h * 33
                        for half in range(2):
                            nc.tensor.matmul(
                                oph[:, half * 512:(half + 1) * 512],
                                xp33[:, base:base + 33],
                                tA[:, half * 512:(half + 1) * 512],
                                start=(J == 0), stop=(J == NBLK - 1),
                            )
                    # ---- divide by denom, relu, max-pool ----
                    # reciprocal of the denom row, folded 1x1024 -> 32x32 so
                    # the DVE reciprocal runs 32 partitions wide
                    denr = mpool.tile([33, NPAD], FP, tag="denr")
                    nc.scalar.copy(denr[32:33, :], oph[32:33, :])
                    den32 = mpool.tile([32, 32], FP, tag="den32")
                    nc.sync.dma_start(den32[:], denr[32:33, :])
                    rec32 = mpool.tile([32, 32], FP, tag="rec32")
                    nc.vector.reciprocal(rec32[:], den32[:])
                    rech = mpool.tile([33, NPAD], FP, tag="rech")
                    nc.sync.dma_start(rech[32:33, :], rec32[:])
                    outh = mpool.tile([32, NPAD], FP, tag="outh")
                    nc.vector.tensor_copy(outh[:], oph[0:32, :])
                    odiv = mpool.tile([32, NPAD], FP, tag="odiv")
                    for half in range(2):
                        rb = ps_misc.tile([32, 512], FP, tag="pm")
                        nc.tensor.matmul(
                            rb[:],
                            c_ones32[32:33, :],
                            rech[32:33, half * 512:(half + 1) * 512],
                            start=True, stop=True,
                        )
                        nc.vector.tensor_tensor(
                            odiv[:, half * 512:(half + 1) * 512],
                            outh[:, half * 512:(half + 1) * 512],
                            rb[:], OPS.mult,
                        )
                    orel = mpool.tile([32, NPAD], FP, tag="orel")
                    nc.vector.tensor_scalar(
                        orel[:], odiv[:], c_bgat[:, h:h + 1], 0.0,
                        OPS.add, OPS.max,
                    )
                    nc.vector.tensor_reduce(
                        c_ph[h][:, g:g + 1], orel[:, 0:N], AX.X, OPS.max
                    )

            # assemble pooled [128, G] from the four per-head tiles (DMA: cross-base)
            for h in range(H):
                nc.sync.dma_start(c_pool[h * 32:(h + 1) * 32, :], c_ph[h][:])

            # ---- LSTM over T steps ----
            # h is stored as h2 = 2h (W_hh/W_clf pre-halved on host);
            # c is stored as c2 = 2c (tanh applied with scale=0.5).
            hprev = lpool.tile([HL, 1], FP, tag="h0")
            cprev = lpool.tile([HL, 1], FP, tag="c0")
            nc.vector.memset(hprev[:], 0.0)
            nc.vector.memset(cprev[:], 0.0)
            for t in range(T):
                # four [64,1] gate psums (i, f, g, o), all base partition 0
                tga = []
                for gate in range(4):
                    psg = ps_misc.tile([HL, 1], FP, tag="pm")
                    nc.tensor.matmul(
                        psg[:], c_wih[:, gate * HL:(gate + 1) * HL],
                        c_pool[:, t:t + 1], start=True, stop=False,
                    )
                    nc.tensor.matmul(
                        psg[:], c_whh[:, gate * HL:(gate + 1) * HL],
                        hprev[:], start=False, stop=True,
                    )
                    tgt = lpool.tile([HL, 1], FP, tag=f"tg{gate}")
                    # gates i,f,o: sigmoid via tanh-half; gate g: plain tanh
                    sc = 1.0 if gate == 2 else 0.5
                    nc.scalar.activation(
                        tgt[:], psg[:], AF.Tanh,
                        bias=c_bls[:, gate:gate + 1], scale=sc,
                    )
                    tga.append(tgt)
                ti, tf, tg_, to = tga
                # v1 = (tf+1)*c2 = 4*sig(f)*c ; v2 = (ti+1)*tg = 2*sig(i)*g
                # c2_new = 2c_new = v1/2 + v2
                v1 = lpool.tile([HL, 1], FP, tag="v1")
                nc.vector.scalar_tensor_tensor(
                    v1[:], tf[:], 1.0, cprev[:], OPS.add, OPS.mult
                )
                v2 = lpool.tile([HL, 1], FP, tag="v2")
                nc.vector.scalar_tensor_tensor(
                    v2[:], ti[:], 1.0, tg_[:], OPS.add, OPS.mult
                )
                cnew = lpool.tile([HL, 1], FP, tag="c0")
                nc.vector.scalar_tensor_tensor(
                    cnew[:], v1[:], 0.5, v2[:], OPS.mult, OPS.add
                )
                tcn = lpool.tile([HL, 1], FP, tag="tcn")
                nc.scalar.activation(tcn[:], cnew[:], AF.Tanh, scale=0.5)
                hnew = lpool.tile([HL, 1], FP, tag="h0")
                # h2 = (to + 1) * tanh(c)
                nc.vector.scalar_tensor_tensor(
                    hnew[:], to[:], 1.0, tcn[:], OPS.add, OPS.mult
                )
                hprev, cprev = hnew, cnew

            ps3 = ps_misc.tile([OUT, 1], FP, tag="pm")
            nc.tensor.matmul(ps3[:], c_wclf[:], hprev[:], start=True, stop=True)
            ysb = lpool.tile([OUT, 1], FP, tag="ysb")
            nc.vector.tensor_tensor(ysb[:], ps3[:], c_bclf[:], OPS.add)
            nc.sync.dma_start(d_y, ysb[:])

    nc.compile()
    return nc


def _host_prep(inputs):
    x = np.asarray(inputs["x"], dtype=np.float32)          # [B, T, N, F]
    ei = np.asarray(inputs["edge_index"])
    W_gat = np.asarray(inputs["W_gat"], dtype=np.float32)  # [16, 128]
    att_src = np.asarray(inputs["att_src"], dtype=np.float32)  # [H, D]
    att_dst = np.asarray(inputs["att_dst"], dtype=np.float32)
    b_gat = np.asarray(inputs["b_gat"], dtype=np.float32)
    W_ih = np.asarray(inputs["W_ih"], dtype=np.float32)    # [256, 128]
    W_hh = np.asarray(inputs["W_hh"], dtype=np.float32)    # [256, 64]
    b_ih = np.asarray(inputs["b_ih"], dtype=np.float32)
    b_hh = np.asarray(inputs["b_hh"], dtype=np.float32)
    W_clf = np.asarray(inputs["W_clf"], dtype=np.float32)  # [8, 64]
    b_clf = np.asarray(inputs["b_clf"], dtype=np.float32)

    bf16 = mybir.dt.np(BF)

    # fold attention vectors: a_s = x @ (W_gat-reshaped @ att_src)
    Wr = W_gat.reshape(F_IN, H, D)
    W_as = np.einsum("fhd,hd->fh", Wr, att_src)            # [16, 4]
    W_ad = np.einsum("fhd,hd->fh", Wr, att_dst)
    w_a8 = np.zeros((F_IN, 128), dtype=np.float32)
    w_a8[:, 32 * np.arange(H)] = W_ad                      # a_d -> partition 32h
    w_a8[:, 32 * np.arange(H) + 1] = W_as                  # a_s -> 32h+1

    # edge counts with self loops, dense [1024, 1024]
    src = ei[0].astype(np.int64)
    dst = ei[1].astype(np.int64)
    Cm = np.zeros((NPAD, NPAD), dtype=np.float32)
    np.add.at(Cm, (src, dst), 1.0)
    Cm[np.arange(N), np.arange(N)] += 1.0                  # self loops
    Cm[NPAD - 1, N:] = 1.0  # dummy edges: keep pad-column denominators finite
    cntmask = (
        Cm.reshape(NBLK, 128, NPAD).transpose(1, 0, 2).reshape(128, NBLK * NPAD)
    ).astype(bf16)

    # x padded per core: [T, NPAD, F] flattened
    xpad = np.zeros((B, T, NPAD, F_IN), dtype=np.float32)
    xpad[:, :, :N, :] = x
    xcore = [xpad[b].reshape(T * NPAD, F_IN).copy() for b in range(B)]

    b_gates = (b_ih + b_hh).astype(np.float32)             # [256]
    bls = np.zeros((HL, 4), dtype=np.float32)
    bls[:, 0] = 0.5 * b_gates[0:64]                        # i (tanh-half trick)
    bls[:, 1] = 0.5 * b_gates[64:128]                      # f
    bls[:, 2] = b_gates[128:192]                           # g
    bls[:, 3] = 0.5 * b_gates[192:256]                     # o

    common = {
        "w_gat": W_gat,
        "ones_row": np.ones((1, 128), dtype=bf16),
        "ones_c32": np.ones((33, 32), dtype=np.float32),
        "w_a8": w_a8,
        "cntmask": cntmask,
        "ident": np.eye(128, dtype=np.float32),
        "b_gat": np.ascontiguousarray(b_gat.reshape(H, 32).T),
        "wih_t": np.ascontiguousarray(W_ih.T),             # [128, 256]
        "whh_t": np.ascontiguousarray(0.5 * W_hh.T),       # [64, 256] (h2 comp)
        "b_lstm": bls,
        "wclf_t": np.ascontiguousarray(0.5 * W_clf.T),     # [64, 8] (h2 comp)
        "b_clf": b_clf.reshape(OUT, 1),
    }
    in_maps = []
    for b in range(B):
        m = dict(common)
        m["x_in"] = xcore[b]
        in_maps.append(m)
    return in_maps


def kernel(**inputs):
    if "nc" not in _CACHE:
        _CACHE["nc"] = _build_nc()
    nc = _CACHE["nc"]
    in_maps = _host_prep(inputs)
    res = run_bass_kernel_spmd(nc, in_maps, core_ids=list(range(B)))
    y = np.stack([r["y"][:, 0] for r in res.results], axis=0)
    return y.astype(np.float32)


if __name__ == "__main__":
    import reference as R

    inp = R.setup_inputs()
    inp = {k: np.asarray(v) for k, v in inp.items()}
    out = kernel(**inp)
    print(out)



# revision 35
# speedup vs baseline: 1.6544x; 1.6544x over previous
"""GAT + global-max-pool + LSTM + Linear kernel for Trainium2 (8 NeuronCores).

Sharding: data-parallel over the batch axis B=8 -> one sequence b per core.
Each core computes the GAT over its 16 graphs (t=0..15), global-max-pools,
runs the LSTM over its sequence locally, and emits one [8] output row.

v2 redesign (engine-balanced):
  - host pre-transposes x -> xT [16, G*1024]; per-graph DMA load (no PE
    transposes on device).
  - one merged fp32r matmul per 128-src-node block J computes xp|a_src
    directly in [m, hd] layout (no xpT + transpose round trip).
  - per head: raw a_dst row broadcast once to 128 partitions (2 fp32r
    matmuls -> adB PSUM); vb_sb = Exp(adB), v5b_sb = Exp(0.2 adB) SBUF
    mirrors feed the DVE fast paths.
  - dense attention tile per (head h, src block J), engine-split:
      A-path (small J): t1 = ACT Exp(adB + a_s), t5 = ACT Exp(.2 adB + .2 a_s),
                        tM = DVE tt-max
      B-path: tE = DVE ts(vb_sb * u) in 4x mode; tM via Pool fused stt or
              DVE ts+tt
      tA = tM * cnt on DVE (2x bf16) or Pool
      agg: oph[33, n] += xp33^T @ tA   (bf16 matmuls, 500-col halves)
  - divide+relu+maxpool fused and software-pipelined two heads deep:
    Pool copies oph -> SBUF (frees the single PSUM buffer), DMA folds the
    denominator row to [40,25], DVE reciprocal, DMA unfold, bf16 PE
    broadcast of rec row, tensor_tensor_reduce chains the max over both
    halves; b_gat applied after the reduce (exact: b is constant over
    nodes).
  - LSTM step g emitted inside graph g+1 (hidden behind its factor
    stage). Gates packed 2-per-matmul, tanh-sigmoid trick with
    per-partition scale columns.
  - real 1000 cols only, gap layout (halves at col offsets 0 and 512).

softmax max-subtraction is dropped: alpha = ex/sum(ex) is invariant to the
per-dst shift and fp32/bf16 exp() of |z| <~ 10 cannot overflow.
"""

import numpy as np

import concourse.bacc as bacc
import concourse.bass as bass
import concourse.mybir as mybir
import concourse.tile as tile
from concourse.bass_utils import run_bass_kernel_spmd

B, T, N, F_IN = 8, 16, 1000, 16
H, D = 4, 32
HD = H * D          # 128
HL = 64
OUT = 8
NEG = 0.2
NPAD = 1024         # padded node count
NBLK = 8            # src blocks of 128
G = T               # graphs per core
NH = 500            # real cols per half
GAP = 512           # col offset stride of halves
BLKC = H * 33 + 4   # xp33 cols per src block (4 head groups + pad)

FP = mybir.dt.float32
FR = mybir.dt.float32r
BF = mybir.dt.bfloat16
AX = mybir.AxisListType
AF = mybir.ActivationFunctionType
OPS = mybir.AluOpType

_CACHE = {}

# engine assignment knobs, per src block J (same for all heads)
A_PATH_J = 0          # J < this: t1/t5 via ACT Exp-with-bias
P2_POOL_J = ()  # fused stt on Pool
P3_POOL_J = ()     # cnt-mult on Pool


def _rv(ap):
    """Real-column view [p, 2, 500] of a gap-layout [p, 1024] AP."""
    return ap.rearrange("p (a b) -> p a b", b=GAP)[:, :, 0:NH]


def _build_nc():
    nc = bacc.Bacc("TRN2", target_bir_lowering=False, debug=False)

    # ---- DRAM I/O ----
    d_xT = nc.dram_tensor("x_t", [F_IN, G * NPAD], FR, kind="ExternalInput").ap()
    d_wall = nc.dram_tensor("w_all", [F_IN, HD + H], FR, kind="ExternalInput").ap()
    d_wad = nc.dram_tensor("w_ad", [F_IN, H], FR, kind="ExternalInput").ap()
    d_cnt = nc.dram_tensor("cntmask", [128, NBLK * NPAD], BF, kind="ExternalInput").ap()
    d_onesb = nc.dram_tensor("ones_bf", [1, 128], BF, kind="ExternalInput").ap()
    d_onesf = nc.dram_tensor("ones_fp", [1, 128], FR, kind="ExternalInput").ap()
    d_bgat = nc.dram_tensor("b_gat", [32, H], FP, kind="ExternalInput").ap()
    d_wih01 = nc.dram_tensor("wih01", [HD, 2 * HL], FP, kind="ExternalInput").ap()
    d_wih23 = nc.dram_tensor("wih23", [HD, 2 * HL], FP, kind="ExternalInput").ap()
    d_whh01 = nc.dram_tensor("whh01", [HL, 2 * HL], FP, kind="ExternalInput").ap()
    d_whh23 = nc.dram_tensor("whh23", [HL, 2 * HL], FP, kind="ExternalInput").ap()
    d_bls = nc.dram_tensor("b_lstm", [2 * HL, 2], FP, kind="ExternalInput").ap()
    d_scl23 = nc.dram_tensor("scl23", [2 * HL, 1], FP, kind="ExternalInput").ap()
    d_wclf = nc.dram_tensor("wclf_t", [HL, OUT], FP, kind="ExternalInput").ap()
    d_bclf = nc.dram_tensor("b_clf", [OUT, 1], FP, kind="ExternalInput").ap()
    d_y = nc.dram_tensor("y", [OUT, 1], FP, kind="ExternalOutput").ap()
    d_dbg = nc.dram_tensor("dbg_pool", [HD, G], FP, kind="ExternalOutput").ap()
    d_dbg2 = nc.dram_tensor("dbg_osb", [33, NPAD], FP, kind="ExternalOutput").ap()
    d_dbg3 = nc.dram_tensor("dbg_rec", [1, NPAD], BF, kind="ExternalOutput").ap()
    d_dbg4 = nc.dram_tensor("dbg_tA", [128, NPAD], BF, kind="ExternalOutput").ap()
    d_dbg5 = nc.dram_tensor("dbg_vb", [128, NPAD], BF, kind="ExternalOutput").ap()
    d_dbg6 = nc.dram_tensor("dbg_fac", [128, NBLK * 8], FP, kind="ExternalOutput").ap()

    with tile.TileContext(nc) as tc:
        with (
            tc.tile_pool(name="const", bufs=1) as cpool,
            tc.tile_pool(name="xtp", bufs=2) as xtp,
            tc.tile_pool(name="fact", bufs=2) as fpool,
            tc.tile_pool(name="bcs", bufs=2) as bpool,
            tc.tile_pool(name="edense", bufs=3) as epool,
            tc.tile_pool(name="divp", bufs=3) as dpool,
            tc.tile_pool(name="lstm", bufs=2) as lpool,
            tc.tile_pool(name="ps_bc", bufs=1, space="PSUM") as ps_bc,
            tc.tile_pool(name="ps_out", bufs=2, space="PSUM") as ps_out,
            tc.tile_pool(name="ps_misc", bufs=2, space="PSUM") as ps_misc,
        ):
            # ---- load constants ----
            c_wall = cpool.tile([F_IN, HD + H], FR, tag="wall")
            nc.sync.dma_start(c_wall[:], d_wall)
            c_wad = cpool.tile([F_IN, H], FR, tag="wad")
            nc.sync.dma_start(c_wad[:], d_wad)
            c_cnt = cpool.tile([128, NBLK * NPAD], BF, tag="cnt")
            nc.sync.dma_start(c_cnt[:], d_cnt)
            c_onesb = cpool.tile([1, 128], BF, tag="onesb")
            nc.sync.dma_start(c_onesb[:], d_onesb)
            c_onesf = cpool.tile([1, 128], FR, tag="onesf")
            nc.sync.dma_start(c_onesf[:], d_onesf)
            c_bgat = cpool.tile([32, H], FP, tag="bgat")
            nc.sync.dma_start(c_bgat[:], d_bgat)
            c_wih01 = cpool.tile([HD, 2 * HL], FP, tag="wih01")
            nc.sync.dma_start(c_wih01[:], d_wih01)
            c_wih23 = cpool.tile([HD, 2 * HL], FP, tag="wih23")
            nc.sync.dma_start(c_wih23[:], d_wih23)
            c_whh01 = cpool.tile([HL, 2 * HL], FP, tag="whh01")
            nc.sync.dma_start(c_whh01[:], d_whh01)
            c_whh23 = cpool.tile([HL, 2 * HL], FP, tag="whh23")
            nc.sync.dma_start(c_whh23[:], d_whh23)
            c_bls = cpool.tile([2 * HL, 2], FP, tag="bls")
            nc.sync.dma_start(c_bls[:], d_bls)
            c_scl23 = cpool.tile([2 * HL, 1], FP, tag="scl23")
            nc.sync.dma_start(c_scl23[:], d_scl23)
            c_wclf = cpool.tile([HL, OUT], FP, tag="wclf")
            nc.sync.dma_start(c_wclf[:], d_wclf)
            c_bclf = cpool.tile([OUT, 1], FP, tag="bclf")
            nc.sync.dma_start(c_bclf[:], d_bclf)

            # persistent: pooled sequence + manually double-buffered xp33
            c_pool = cpool.tile([HD, G], FP, tag="pooled")
            xp33s = []
            for i in range(2):
                xp_t = cpool.tile([128, NBLK * BLKC], BF, tag=f"xp33_{i}")
                for J in range(NBLK):
                    nc.vector.memset(
                        xp_t[:, J * BLKC:J * BLKC + H * 33].rearrange(
                            "p (h q) -> p h q", q=33
                        )[:, :, 32:33],
                        1.0,
                    )
                xp33s.append(xp_t)

            hprev = lpool.tile([HL, 1], FP, tag="h0")
            cprev = lpool.tile([HL, 1], FP, tag="c0")
            nc.vector.memset(hprev[:], 0.0)
            nc.vector.memset(cprev[:], 0.0)
            lstm_state = [hprev, cprev]

            def emit_lstm(g):
                hp, cp = lstm_state
                psg01 = ps_misc.tile([2 * HL, 1], FP, tag="pm")
                nc.tensor.matmul(
                    psg01[:], c_wih01[:],
                    c_pool[:, g:g + 1], start=True, stop=False,
                )
                nc.tensor.matmul(
                    psg01[:], c_whh01[:], hp[:],
                    start=False, stop=True,
                )
                psg23 = ps_misc.tile([2 * HL, 1], FP, tag="pm")
                nc.tensor.matmul(
                    psg23[:], c_wih23[:],
                    c_pool[:, g:g + 1], start=True, stop=False,
                )
                nc.tensor.matmul(
                    psg23[:], c_whh23[:], hp[:],
                    start=False, stop=True,
                )
                tg01 = lpool.tile([2 * HL, 1], FP, tag="tg01")
                nc.scalar.activation(
                    tg01[:], psg01[:], AF.Tanh, bias=c_bls[:, 0:1], scale=0.5,
                )
                tg23 = lpool.tile([2 * HL, 1], FP, tag="tg23")
                nc.scalar.activation(
                    tg23[:], psg23[:], AF.Tanh, bias=c_bls[:, 1:2],
                    scale=c_scl23[:, 0:1],
                )
                tf0 = lpool.tile([HL, 1], FP, tag="tf0")
                nc.sync.dma_start(tf0[:], tg01[HL:2 * HL, :])
                to0 = lpool.tile([HL, 1], FP, tag="to0")
                nc.sync.dma_start(to0[:], tg23[HL:2 * HL, :])
                # v1 = (tf+1)*c2 ; v2 = (ti+1)*tg ; c2' = v1/2 + v2
                v1 = lpool.tile([HL, 1], FP, tag="v1")
                nc.vector.scalar_tensor_tensor(
                    v1[:], tf0[:], 1.0, cp[:], OPS.add, OPS.mult
                )
                v2 = lpool.tile([HL, 1], FP, tag="v2")
                nc.vector.scalar_tensor_tensor(
                    v2[:], tg01[0:HL, :], 1.0, tg23[0:HL, :], OPS.add, OPS.mult
                )
                cnew = lpool.tile([HL, 1], FP, tag="c0")
                nc.vector.scalar_tensor_tensor(
                    cnew[:], v1[:], 0.5, v2[:], OPS.mult, OPS.add
                )
                tcn = lpool.tile([HL, 1], FP, tag="tcn")
                nc.scalar.activation(tcn[:], cnew[:], AF.Tanh, scale=0.5)
                hnew = lpool.tile([HL, 1], FP, tag="h0")
                nc.vector.scalar_tensor_tensor(
                    hnew[:], to0[:], 1.0, tcn[:], OPS.add, OPS.mult
                )
                lstm_state[0], lstm_state[1] = hnew, cnew

            def emit_bcast(ad_row, h):
                """Broadcast a_d row h to 128 partitions; SBUF mirrors."""
                adB = ps_bc.tile([128, NPAD], FP, tag="adB")
                for half in range(2):
                    sl = slice(half * GAP, half * GAP + NH)
                    nc.tensor.matmul(
                        adB[:, sl], c_onesf[:],
                        ad_row[:, sl],
                        start=True, stop=True,
                    )
                vb_sb = bpool.tile([128, NPAD], BF, tag=f"vb_sb{h}")
                nc.scalar.activation(_rv(vb_sb[:]), _rv(adB[:]), AF.Exp)
                v5b_sb = bpool.tile([128, NPAD], BF, tag=f"v5b_sb{h}")
                nc.scalar.activation(_rv(v5b_sb[:]), _rv(adB[:]), AF.Exp, scale=NEG)
                return adB, vb_sb, v5b_sb

            for g in range(G):
                xp33 = xp33s[g % 2]
                # ---- load xT for this graph ----
                xT = xtp.tile([F_IN, NPAD], FR, tag="xT")
                nc.sync.dma_start(xT[:], d_xT[:, g * NPAD:(g + 1) * NPAD])
                xTr = xT[:]

                # ---- a_dst rows (raw, fp32); one [1, NPAD] tile per head
                # (matmul rhs requires base partition 0) ----
                ad_sb = fpool.tile([H, NPAD], FR, tag="ad_sb")
                for half in range(2):
                    pad_ = ps_misc.tile([H, GAP], FP, tag="pm")
                    nc.tensor.matmul(
                        pad_[:, 0:NH], c_wad[:],
                        xTr[:, half * NH:half * NH + NH],
                        start=True, stop=True,
                    )
                    nc.scalar.activation(
                        ad_sb[:, half * GAP:half * GAP + NH],
                        pad_[:, 0:NH], AF.Copy,
                    )
                nc.vector.memset(ad_sb[:, NH:GAP].bitcast(mybir.dt.uint32), 0)
                nc.vector.memset(ad_sb[:, GAP + NH:NPAD].bitcast(mybir.dt.uint32), 0)
                ad_rows = []
                for h in range(H):
                    ad_h = fpool.tile([1, NPAD], FR, tag=f"ad_h{h}")
                    nc.sync.dma_start(ad_h[:], ad_sb[h:h + 1, :])
                    ad_rows.append(ad_h)

                # ---- per src block: xp | a_src factors, plus all four
                # head broadcasts interleaved to keep the PE streaming ----
                # c_fac cols per J: [0:4] = a_s (A-path) or exp(a_s) (B-path)
                #                   [4:8] = 0.2*a_s or exp(0.2*a_s)
                c_fac = fpool.tile([128, NBLK * 8], FP, tag="c_fac")
                bcast = [None] * H

                def emit_pxa(J):
                    pxa = ps_misc.tile([128, HD + H], FP, tag="pm")
                    nc.tensor.matmul(
                        pxa[:], xTr[:, J * 128:(J + 1) * 128],
                        c_wall[:],
                        start=True, stop=True,
                    )
                    base = J * BLKC
                    nc.vector.tensor_copy(
                        xp33[:, base:base + H * 33].rearrange(
                            "p (h q) -> p h q", q=33
                        )[:, :, 0:32],
                        pxa[:, 0:HD].rearrange("p (h q) -> p h q", q=32),
                    )
                    if J < PRELU_J:
                        nc.vector.tensor_copy(
                            c_fac[:, J * 8:J * 8 + 4], pxa[:, HD:HD + H],
                        )
                    else:
                        nc.scalar.activation(
                            c_fac[:, J * 8:J * 8 + 4], pxa[:, HD:HD + H],
                            AF.Exp, scale=1.0,
                        )
                        nc.scalar.activation(
                            c_fac[:, J * 8 + 4:J * 8 + 8], pxa[:, HD:HD + H],
                            AF.Exp, scale=NEG,
                        )

                emit_pxa(0)
                emit_pxa(1)
                for h in range(H):
                    bcast[h] = emit_bcast(ad_rows[h], h)
                    if 2 + h < NBLK:
                        emit_pxa(2 + h)
                emit_pxa(6)
                emit_pxa(7)
                if g > 0:
                    emit_lstm(g - 1)

                # ---- heads: software-pipelined divide path ----
                # stage state carried across head iterations
                pending = {}   # h -> dict of tiles for deferred stages
                for h in range(H):
                    adB_ps, vb_sb, v5b_sb = bcast[h]

                    oph = ps_out.tile([33, NPAD], FP, tag="oph")
                    for J in range(NBLK):
                        fcol = c_fac[:, J * 8 + h:J * 8 + h + 1]
                        f5col = c_fac[:, J * 8 + 4 + h:J * 8 + 5 + h]
                        cntJ = c_cnt[:, J * NPAD:(J + 1) * NPAD]
                        tA = epool.tile([128, NPAD], BF, tag="tA")
                        if J < PRELU_J:
                            # w = lrelu(a_d[n] + a_s[m]); tM = exp(w)
                            tw = epool.tile([128, NPAD], mybir.dt.float16,
                                            tag="tw")
                            nc.scalar.activation(
                                _rv(tw[:]), _rv(adB_ps[:]), AF.Prelu,
                                bias=fcol, scale=1.0, alpha=NEG,
                            )
                            tM = epool.tile([128, NPAD], BF, tag="tM")
                            nc.scalar.activation(
                                _rv(tM[:]), _rv(tw[:]), AF.Exp,
                            )
                            eng = nc.gpsimd if J in P3_POOL_J else nc.vector
                            eng.tensor_tensor(
                                _rv(tA[:]), _rv(tM[:]), _rv(cntJ), OPS.mult,
                            )
                        else:
                            t1 = epool.tile([128, NPAD], BF, tag="t1")
                            nc.vector.tensor_scalar(
                                _rv(t1[:]), _rv(vb_sb[:]), fcol, None, OPS.mult,
                            )
                            t5 = epool.tile([128, NPAD], BF, tag="t5")
                            nc.vector.tensor_scalar(
                                _rv(t5[:]), _rv(v5b_sb[:]), f5col, None,
                                OPS.mult,
                            )
                            tM = epool.tile([128, NPAD], BF, tag="tM")
                            nc.vector.tensor_tensor(
                                _rv(tM[:]), _rv(t5[:]), _rv(t1[:]), OPS.max
                            )
                            nc.vector.tensor_tensor(
                                _rv(tA[:]), _rv(tM[:]), _rv(cntJ), OPS.mult,
                            )

                        if g == 0 and h == 0 and J == 0:
                            nc.sync.dma_start(_rv(d_dbg4), _rv(tA[:]))
                            nc.sync.dma_start(_rv(d_dbg5), _rv(vb_sb[:]))
                            nc.sync.dma_start(d_dbg6, c_fac[:])
                        base = J * BLKC + h * 33
                        for half in range(2):
                            sl = slice(half * GAP, half * GAP + NH)
                            nc.tensor.matmul(
                                oph[:, sl], xp33[:, base:base + 33], tA[:, sl],
                                start=(J == 0), stop=(J == NBLK - 1),
                            )

                    # stage_drain(h): free the single oph PSUM buffer
                    o_sb = dpool.tile([33, NPAD], FP, tag="o_sb")
                    nc.vector.tensor_copy(_rv(o_sb[:]), _rv(oph[:]))
                    if g == 0 and h == 0:
                        nc.sync.dma_start(_rv(d_dbg2), _rv(o_sb[:]))
                    den40 = dpool.tile([40, 25], FP, tag="den40")
                    for half in range(2):
                        nc.sync.dma_start(
                            den40[half * 20:(half + 1) * 20, :],
                            o_sb[32:33, half * GAP:half * GAP + NH],
                        )
                    pending[h] = {"o_sb": o_sb, "den40": den40}

                    if g == 0 and h == 1 and "recrow" in pending.get(0, {}):
                        pass
                    # stage_recip(h-1)
                    if h - 1 in pending:
                        st = pending[h - 1]
                        rec40 = dpool.tile([40, 25], BF, tag="rec40")
                        with nc.allow_low_precision(reason="bf16 rec ok"):
                            nc.vector.reciprocal(rec40[:], st["den40"][:])
                        recrow = dpool.tile([1, NPAD], BF, tag="recrow")
                        for half in range(2):
                            nc.sync.dma_start(
                                recrow[:, half * GAP:half * GAP + NH],
                                rec40[half * 20:(half + 1) * 20, :],
                            )
                        st["recrow"] = recrow
                        if g == 0 and h - 1 == 0:
                            nc.sync.dma_start(_rv(d_dbg3), _rv(recrow[:]))

                    # stage_finish(h-2)
                    if h - 2 in pending:
                        _emit_finish(nc, ps_misc, dpool, pending.pop(h - 2),
                                     c_onesb, c_bgat, c_pool, h - 2, g)

                # epilogue: flush heads 2 and 3
                st = pending[H - 1]
                rec40 = dpool.tile([40, 25], BF, tag="rec40")
                with nc.allow_low_precision(reason="bf16 rec ok"):
                    nc.vector.reciprocal(rec40[:], st["den40"][:])
                recrow = dpool.tile([1, NPAD], BF, tag="recrow")
                for half in range(2):
                    nc.sync.dma_start(
                        recrow[:, half * GAP:half * GAP + NH],
                        rec40[half * 20:(half + 1) * 20, :],
                    )
                st["recrow"] = recrow
                _emit_finish(nc, ps_misc, dpool, pending.pop(H - 2),
                             c_onesb, c_bgat, c_pool, H - 2, g)
                _emit_finish(nc, ps_misc, dpool, pending.pop(H - 1),
                             c_onesb, c_bgat, c_pool, H - 1, g)

            emit_lstm(G - 1)
            hp = lstm_state[0]
            ps3 = ps_misc.tile([OUT, 1], FP, tag="pm")
            nc.tensor.matmul(
                ps3[:], c_wclf[:], hp[:],
                start=True, stop=True,
            )
            ysb = lpool.tile([OUT, 1], FP, tag="ysb")
            nc.vector.tensor_tensor(ysb[:], ps3[:], c_bclf[:], OPS.add)
            nc.sync.dma_start(d_y, ysb[:])
            nc.sync.dma_start(d_dbg, c_pool[:])

    nc.compile()
    return nc


def _emit_finish(nc, ps_misc, dpool, st, c_onesb, c_bgat, c_pool, h, g):
    """rec-row broadcast + fused divide/maxpool + pooled write for head h."""
    o_sb, recrow = st["o_sb"], st["recrow"]
    scr = dpool.tile([32, NPAD], BF, tag="scr")
    macc = dpool.tile([32, 1], FP, tag="macc")
    for half in range(2):
        sl = slice(half * GAP, half * GAP + NH)
        rb = ps_misc.tile([32, GAP], FP, tag="pm")
        nc.tensor.matmul(
            rb[:, 0:NH], c_onesb[:, 0:32], recrow[:, sl],
            start=True, stop=True,
        )
        nc.vector.tensor_tensor(
            scr[:, sl], o_sb[0:32, sl], rb[:, 0:NH], OPS.mult
        )
    nc.vector.tensor_reduce(macc[:], _rv(scr[:]), AX.XY, OPS.max)
    pooled_h = dpool.tile([32, 1], FP, tag="pooled_h")
    nc.vector.tensor_scalar(
        pooled_h[:], macc[:], c_bgat[:, h:h + 1], 0.0,
        OPS.add, OPS.max,
    )
    nc.sync.dma_start(c_pool[h * 32:(h + 1) * 32, g:g + 1], pooled_h[:])


def _host_prep(inputs):
    x = np.asarray(inputs["x"], dtype=np.float32)          # [B, T, N, F]
    ei = np.asarray(inputs["edge_index"])
    W_gat = np.asarray(inputs["W_gat"], dtype=np.float32)  # [16, 128]
    att_src = np.asarray(inputs["att_src"], dtype=np.float32)  # [H, D]
    att_dst = np.asarray(inputs["att_dst"], dtype=np.float32)
    b_gat = np.asarray(inputs["b_gat"], dtype=np.float32)
    W_ih = np.asarray(inputs["W_ih"], dtype=np.float32)    # [256, 128]
    W_hh = np.asarray(inputs["W_hh"], dtype=np.float32)    # [256, 64]
    b_ih = np.asarray(inputs["b_ih"], dtype=np.float32)
    b_hh = np.asarray(inputs["b_hh"], dtype=np.float32)
    W_clf = np.asarray(inputs["W_clf"], dtype=np.float32)  # [8, 64]
    b_clf = np.asarray(inputs["b_clf"], dtype=np.float32)

    bf16 = mybir.dt.np(BF)

    def round_fr(x):
        u = np.ascontiguousarray(x, dtype=np.float32).view(np.uint32)
        r = ((u.astype(np.uint64) + 0x800) & 0xFFFFF000).astype(np.uint32)
        return r.view(np.float32)

    # fold attention vectors: a_s = x @ (W_gat-reshaped @ att_src)
    Wr = W_gat.reshape(F_IN, H, D)
    W_as = np.einsum("fhd,hd->fh", Wr, att_src)            # [16, 4]
    W_ad = np.einsum("fhd,hd->fh", Wr, att_dst)
    w_all = np.concatenate([W_gat, W_as], axis=1)          # [16, 132]

    # edge counts with self loops; gap layout [128, 8*(512+512)],
    # halves hold real cols 0:500 / 500:1000 at offsets 0 / 512
    src = ei[0].astype(np.int64)
    dst = ei[1].astype(np.int64)
    Cm = np.zeros((NPAD, N), dtype=np.float32)
    np.add.at(Cm, (src, dst), 1.0)
    Cm[np.arange(N), np.arange(N)] += 1.0                  # self loops
    cnt4 = np.zeros((NBLK, 128, 2, GAP), dtype=np.float32)
    CmJ = Cm.reshape(NBLK, 128, N)
    cnt4[:, :, 0, 0:NH] = CmJ[:, :, 0:NH]
    cnt4[:, :, 1, 0:NH] = CmJ[:, :, NH:N]
    cntmask = (
        cnt4.reshape(NBLK, 128, NPAD).transpose(1, 0, 2).reshape(128, NBLK * NPAD)
    ).astype(bf16)

    # x pre-transposed per core: [F, G*NPAD]
    xpad = np.zeros((B, T, NPAD, F_IN), dtype=np.float32)
    xpad[:, :, :N, :] = x
    xts = [
        round_fr(xpad[b].transpose(2, 0, 1).reshape(F_IN, T * NPAD))
        for b in range(B)
    ]

    b_gates = (b_ih + b_hh).astype(np.float32)             # [256]
    bls = np.zeros((2 * HL, 2), dtype=np.float32)
    bls[:, 0] = 0.5 * b_gates[0:128]                       # i, f (tanh trick)
    bls[0:HL, 1] = b_gates[128:192]                        # g
    bls[HL:2 * HL, 1] = 0.5 * b_gates[192:256]             # o
    scl23 = np.zeros((2 * HL, 1), dtype=np.float32)
    scl23[0:HL, 0] = 1.0
    scl23[HL:2 * HL, 0] = 0.5

    common = {
        "w_all": round_fr(w_all),
        "w_ad": round_fr(W_ad),
        "cntmask": cntmask,
        "ones_bf": np.ones((1, 128), dtype=bf16),
        "ones_fp": np.ones((1, 128), dtype=np.float32),
        "b_gat": np.ascontiguousarray(b_gat.reshape(H, 32).T),
        "wih01": np.ascontiguousarray(W_ih[0:128, :].T),       # [128, 128]
        "wih23": np.ascontiguousarray(W_ih[128:256, :].T),
        "whh01": np.ascontiguousarray(0.5 * W_hh[0:128, :].T),  # [64, 128]
        "whh23": np.ascontiguousarray(0.5 * W_hh[128:256, :].T),
        "b_lstm": bls,
        "scl23": scl23,
        "wclf_t": np.ascontiguousarray(0.5 * W_clf.T),     # [64, 8] (h2 comp)
        "b_clf": b_clf.reshape(OUT, 1),
    }
    in_maps = []
    for b in range(B):
        m = dict(common)
        m["x_t"] = xts[b]
        in_maps.append(m)
    return in_maps


def kernel(**inputs):
    if "nc" not in _CACHE:
        _CACHE["nc"] = _build_nc()
    nc = _CACHE["nc"]
    in_maps = _host_prep(inputs)
    res = run_bass_kernel_spmd(nc, in_maps, core_ids=list(range(B)))
    y = np.stack([r["y"][:, 0] for r in res.results], axis=0)
    return y.astype(np.float32)


if __name__ == "__main__":
    import reference as R

    inp = R.setup_inputs()
    inp = {k: np.asarray(v) for k, v in inp.items()}
    out = kernel(**inp)
    print(out)
